# revision 9
# baseline (speedup 1.0000x reference)
"""GAT (3-layer, no-LeakyReLU) on 8 Trainium2 NeuronCores — v3.

Math: softmax is separable (no LeakyReLU): with aj[n,h] = <h[n,h,:],
att[h,C:]> and u = exp(aj),
    out[d] = sum_{e: dst=d} u[src_e]*h[src_e] / sum_e u[src_e]
(the ai[dst] term cancels inside the per-destination softmax).

Per layer, per core (nodes sharded by destination):
  1. dense:  h = x @ W.T; u = exp(x @ wj); G row = [u*h | u] (640-wide)
  2. AllGather G -> replicated table in (pair-shared) HBM
  3. per dst-block of 128 nodes: dma_gather G[src] rows, one-hot matmul
     accumulates F = oh.T @ u*h and S1 = oh.T @ u
  4. out = F / S1; transpose to xT for the next layer's dense.

v3 over the v1 baseline:
  - Host LPT permutation balances in-edges per (core, dst-block) bin:
    tiles per layer drop 340 -> 320 and skew padding disappears.
  - L3 computes F and S1 in ONE 101-col matmul (row = [u*h | u]).
  - DVE ops fused: u*h and the output normalization use a single
    broadcast tensor_mul over [P,4,128] views; transposed blocks land in
    one [P,512] PSUM tile copied with one op.
  - Deeper gather pipeline (gt bufs 8).
"""

import numpy as np
import ml_dtypes

N = 20000
E = 320000
NCORES = 8
NSH = 2500            # real nodes per core
NSHP = 2560           # padded to 20 x 128
P = 128
RT = NSHP // P        # row tiles / dst blocks per core = 20
GB = 8                # gather batch: tiles per dma_gather (1024-desc scratch limit)

# layer configs: row = [u*h (H*C) | u (H)] padded to GW
L1 = dict(H=4, C=128, KB=1, GW=640, UO=512)
L2 = dict(H=4, C=128, KB=4, GW=640, UO=512)
L3 = dict(H=1, C=100, KB=4, GW=256, UO=100)

BF16 = ml_dtypes.bfloat16


def _balance_nodes(edge_index):
    """LPT: assign nodes to (core, block) bins balancing in-edge counts."""
    import heapq

    indeg = np.bincount(edge_index[1], minlength=N)
    order = np.argsort(-indeg, kind="stable")
    heap = [(0, 0, b) for b in range(NCORES * RT)]  # (edges, nnodes, bin)
    heapq.heapify(heap)
    node_bin = np.zeros(N, np.int64)
    node_pos = np.zeros(N, np.int64)
    for n in order:
        while True:
            e, c, b = heapq.heappop(heap)
            if c < P:
                break
        node_bin[n] = b
        node_pos[n] = c
        heapq.heappush(heap, (e + int(indeg[n]), c + 1, b))
    return node_bin // RT, node_bin % RT, node_pos


def _preprocess(edge_index):
    """Permute nodes; per-core gather indices + one-hot tiles grouped by
    dst block."""
    ncore, nblk, npos = _balance_nodes(edge_index)
    lrow = nblk * P + npos                       # local row within core

    src = edge_index[0].astype(np.int64)
    dst = edge_index[1].astype(np.int64)
    kd = ncore[dst]
    bd = nblk[dst]
    pd = npos[dst]
    rs = ncore[src] * NSHP + lrow[src]           # table row of source

    order = np.lexsort((rs, bd, kd))
    kd_s, bd_s, pd_s, rs_s = (a[order] for a in (kd, bd, pd, rs))

    cnt = np.zeros((NCORES, RT), np.int64)
    for k in range(NCORES):
        cnt[k] = np.bincount(bd_s[kd_s == k], minlength=RT)

    tb = np.ceil(cnt / P).astype(np.int64).max(axis=0)    # [RT]
    T = int(tb.sum())
    NB = (T + GB - 1) // GB

    block_of_tile = []
    first = []
    last = []
    grp_off = np.zeros(RT, np.int64)
    acc = 0
    for b in range(RT):
        grp_off[b] = acc
        for t in range(tb[b]):
            block_of_tile.append(b)
            first.append(t == 0)
            last.append(t == tb[b] - 1)
        acc += tb[b]
    block_of_tile = np.array(block_of_tile)
    first = np.array(first)
    last = np.array(last)

    idxs_all = np.zeros((NCORES, T * P), np.int64)
    onehot_all = np.zeros((NCORES, T, P, P), BF16)
    for k in range(NCORES):
        m = kd_s == k
        bk, pk, rk = bd_s[m], pd_s[m], rs_s[m]
        off = np.concatenate([[0], np.cumsum(cnt[k])])
        for b in range(RT):
            e0, e1 = off[b], off[b + 1]
            n_e = e1 - e0
            if n_e == 0:
                continue
            slots = grp_off[b] * P + np.arange(n_e)
            idxs_all[k, slots] = rk[e0:e1]
            onehot_all[k, slots // P, slots % P, pk[e0:e1]] = 1.0

    gbc = GB * P // 16
    idx_wrapped = np.zeros((NCORES, 16, NB * gbc), np.int16)
    for g in range(NB):
        i0 = g * GB * P
        n_i = min(GB * P, T * P - i0)
        chunk = idxs_all[:, i0:i0 + n_i].astype(np.int16)
        idx_wrapped[:, :, g * gbc: g * gbc + n_i // 16] = (
            chunk.reshape(NCORES, n_i // 16, 16).transpose(0, 2, 1)
        )
    idx_rep = np.tile(idx_wrapped, (1, 8, 1))

    oh_b = np.zeros((NCORES, NB, P, GB * P), BF16)
    for g in range(NB):
        nt = min(GB, T - g * GB)
        chunk = onehot_all[:, g * GB:g * GB + nt]
        oh_b[:, g, :, :nt * P] = chunk.transpose(0, 2, 1, 3).reshape(
            NCORES, P, nt * P)

    return dict(
        T=T, NB=NB, tb=tb,
        block_of_tile=block_of_tile, first=first, last=last,
        idxs=idx_rep, onehot=oh_b,
        ncore=ncore, nblk=nblk, npos=npos,
    )


def _build_program(ep):
    import concourse.bacc as bacc
    import concourse.mybir as mybir
    import concourse.tile as tile
    from concourse.masks import make_identity

    T, NB = ep["T"], ep["NB"]
    bot, first, last = ep["block_of_tile"], ep["first"], ep["last"]
    f32, bf16, i16 = mybir.dt.float32, mybir.dt.bfloat16, mybir.dt.int16
    gbc = GB * P // 16

    nc = bacc.Bacc("TRN2", target_bir_lowering=False, debug=False,
                   num_devices=NCORES, num_swdge_queues=4)

    # ---- I/O ----
    xT_in = nc.dram_tensor("xT", [P, NSHP], bf16, kind="ExternalInput")
    w1_in = nc.dram_tensor("w1", [P, 512], bf16, kind="ExternalInput")
    wj1_in = nc.dram_tensor("wj1", [P, 4], bf16, kind="ExternalInput")
    w2_in = nc.dram_tensor("w2", [P, 4, 512], bf16, kind="ExternalInput")
    wj2_in = nc.dram_tensor("wj2", [P, 4, 4], bf16, kind="ExternalInput")
    w3_in = nc.dram_tensor("w3", [P, 4, 101], bf16, kind="ExternalInput")
    oh_in = nc.dram_tensor("onehot", [NB, P, GB * P], bf16,
                           kind="ExternalInput")
    idx_in = nc.dram_tensor("idxs", [P, NB * gbc], i16, kind="ExternalInput")
    out_d = nc.dram_tensor("out", [NSHP, 100], f32, kind="ExternalOutput")

    # ---- internal DRAM ----
    ag = [nc.dram_tensor(f"ag{i}", [NSHP, L["GW"]], bf16)
          for i, L in enumerate((L1, L2, L3))]
    table = [nc.dram_tensor(f"table{i}", [NCORES * NSHP, L["GW"]], bf16,
                            addr_space="Shared")
             for i, L in enumerate((L1, L2, L3))]

    with tile.TileContext(nc, num_cores=NCORES) as tc:
        with (
            tc.tile_pool(name="const", bufs=1) as cp,
            tc.tile_pool(name="sb", bufs=2) as sb,
            tc.tile_pool(name="gat", bufs=8) as gp,
            tc.tile_pool(name="small", bufs=4) as sp,
            tc.tile_pool(name="psum", bufs=2, space="PSUM") as psA,
        ):
            ident = cp.tile([P, P], bf16, tag="ident")
            make_identity(nc, ident[:])
            idx_sb = cp.tile([P, NB * gbc], i16, tag="idx")
            nc.sync.dma_start(out=idx_sb[:], in_=idx_in[:, :])
            w_sb = []
            wj_sb = []
            for i, (L, wt, wj) in enumerate(
                ((L1, w1_in, wj1_in), (L2, w2_in, wj2_in), (L3, w3_in, None))
            ):
                nout = 512 if i < 2 else 101
                w = cp.tile([P, L["KB"], nout], bf16, tag=f"w{i}", name=f"w{i}")
                nc.sync.dma_start(out=w[:, 0, :] if i == 0 else w[:],
                                  in_=wt.ap())
                w_sb.append(w)
                if wj is not None:
                    wjt = cp.tile([P, L["KB"], L["H"]], bf16, tag=f"wj{i}",
                                  name=f"wj{i}")
                    nc.sync.dma_start(out=wjt[:, 0, :] if i == 0 else wjt[:],
                                      in_=wj.ap())
                    wj_sb.append(wjt)
                else:
                    wj_sb.append(None)

            xT1 = cp.tile([P, 1, NSHP], bf16, tag="xT1")
            nc.sync.dma_start(out=xT1[:, 0, :], in_=xT_in.ap())
            xT2 = cp.tile([P, 4, NSHP], bf16, tag="xT2", name="xT2")
            xT3 = cp.tile([P, 4, NSHP], bf16, tag="xT3", name="xT3")
            xT_next = [None, xT2, xT3]

            for li, L in enumerate((L1, L2, L3)):
                H, C, KB, GW, UO = L["H"], L["C"], L["KB"], L["GW"], L["UO"]
                is_l3 = li == 2
                G_sb = sb.tile([P, RT, GW], bf16, tag="G", bufs=1)
                xT = xT1 if li == 0 else xT_next[li]

                # ---------- dense + u ----------
                for r in range(RT):
                    if is_l3:
                        # single matmul: w3 holds [W | wj] = 101 cols
                        ph = psA.tile([P, 101], f32, tag="ph")
                        for kb in range(KB):
                            nc.tensor.matmul(ph[:], xT[:, kb, r * P:(r + 1) * P],
                                             w_sb[li][:, kb, :],
                                             start=(kb == 0),
                                             stop=(kb == KB - 1))
                        u = sp.tile([P, 1], f32, tag="u")
                        nc.scalar.activation(u[:], ph[:, 100:101],
                                             mybir.ActivationFunctionType.Exp)
                        nc.vector.tensor_scalar_mul(
                            G_sb[:, r, 0:100], ph[:, 0:100], u[:, 0:1])
                        nc.vector.tensor_copy(out=G_sb[:, r, 100:101],
                                              in_=u[:])
                    else:
                        ph = psA.tile([P, 512], f32, tag="ph")
                        pa = psA.tile([P, H], f32, tag="pa", bufs=1)
                        for kb in range(KB):
                            lhsT = xT[:, kb, r * P:(r + 1) * P]
                            nc.tensor.matmul(ph[:], lhsT, w_sb[li][:, kb, :],
                                             start=(kb == 0),
                                             stop=(kb == KB - 1))
                            nc.tensor.matmul(pa[:], lhsT, wj_sb[li][:, kb, :],
                                             start=(kb == 0),
                                             stop=(kb == KB - 1))
                        u = sp.tile([P, H], f32, tag="u")
                        nc.scalar.activation(u[:], pa[:],
                                             mybir.ActivationFunctionType.Exp)
                        # u*h in ONE broadcast multiply over [P,H,C]
                        nc.vector.tensor_mul(
                            out=G_sb[:, r, 0:512].rearrange(
                                "p (h c) -> p h c", c=C),
                            in0=ph[:].rearrange("p (h c) -> p h c", c=C),
                            in1=u[:].unsqueeze(2).to_broadcast([P, H, C]))
                        nc.vector.tensor_copy(out=G_sb[:, r, UO:UO + H],
                                              in_=u[:])
                    nc.sync.dma_start(
                        out=ag[li][r * P:(r + 1) * P, :],
                        in_=G_sb[:, r, :])

                # ---------- exchange ----------
                nc.gpsimd.collective_compute(
                    "AllGather", mybir.AluOpType.bypass,
                    replica_groups=[list(range(NCORES))],
                    ins=[ag[li].ap().opt()],
                    outs=[table[li].ap().opt()])

                # ---------- edge aggregation ----------
                pF = pS = None
                for g in range(NB):
                    s0 = g * GB
                    nt = min(GB, T - s0)
                    gt = gp.tile([P, GB, GW], bf16, tag="gt")
                    nc.gpsimd.dma_gather(
                        gt[:, :nt, :], table[li].ap(),
                        idx_sb[:, g * gbc: g * gbc + nt * P // 16],
                        nt * P, nt * P, GW, queue_num=g % 4,
                        single_packet=False)
                    oh = gp.tile([P, GB, P], bf16, tag="oh", bufs=10)
                    nc.sync.dma_start(
                        out=oh[:, :nt, :],
                        in_=oh_in[g, :, :nt * P].rearrange(
                            "p (t d) -> p t d", d=P))
                    for tl in range(nt):
                        ti = s0 + tl
                        b = bot[ti]
                        if first[ti]:
                            pF = psA.tile([P, 101 if is_l3 else 512], f32,
                                          tag="pF")
                            if not is_l3:
                                pS = psA.tile([P, H], f32, tag="pS")
                        if is_l3:
                            nc.tensor.matmul(pF[:], oh[:, tl, :],
                                             gt[:, tl, 0:101],
                                             start=bool(first[ti]),
                                             stop=bool(last[ti]))
                        else:
                            nc.tensor.matmul(pF[:], oh[:, tl, :],
                                             gt[:, tl, 0:512],
                                             start=bool(first[ti]),
                                             stop=bool(last[ti]))
                            nc.tensor.matmul(pS[:], oh[:, tl, :],
                                             gt[:, tl, UO:UO + H],
                                             start=bool(first[ti]),
                                             stop=bool(last[ti]))
                        if not last[ti]:
                            continue
                        # ---------- block evacuation ----------
                        if is_l3:
                            nc.vector.tensor_add(
                                out=pF[:], in0=pF[:],
                                in1=G_sb[:, b, 0:101])          # self loop
                            s1c = sp.tile([P, 1], f32, tag="s1c")
                            nc.vector.tensor_scalar_max(
                                s1c[:], pF[:, 100:101], 1e-30)
                            rec = sp.tile([P, 1], f32, tag="rec")
                            nc.vector.reciprocal(rec[:], s1c[:])
                            o3 = sb.tile([P, 100], f32, tag="o3")
                            nc.vector.tensor_scalar_mul(
                                o3[:], pF[:, 0:100], rec[:, 0:1])
                            nc.sync.dma_start(
                                out=out_d[b * P:(b + 1) * P, :], in_=o3[:])
                        else:
                            nc.vector.tensor_add(
                                out=pF[:], in0=pF[:],
                                in1=G_sb[:, b, 0:512])          # self loop
                            nc.vector.tensor_add(
                                out=pS[:], in0=pS[:],
                                in1=G_sb[:, b, UO:UO + H])
                            s1c = sp.tile([P, H], f32, tag="s1c")
                            nc.vector.tensor_scalar_max(s1c[:], pS[:], 1e-30)
                            rec = sp.tile([P, H], f32, tag="rec")
                            nc.vector.reciprocal(rec[:], s1c[:])
                            ob = sb.tile([P, 512], bf16, tag="ob")
                            nc.vector.tensor_mul(
                                out=ob[:].rearrange("p (h c) -> p h c", c=C),
                                in0=pF[:].rearrange("p (h c) -> p h c", c=C),
                                in1=rec[:].unsqueeze(2).to_broadcast(
                                    [P, H, C]))
                            pt = psA.tile([P, 4, P], bf16, tag="pt", bufs=1)
                            for fb in range(4):
                                nc.tensor.transpose(
                                    pt[:, fb, :], ob[:, fb * P:(fb + 1) * P],
                                    ident[:])
                            nc.vector.tensor_copy(
                                out=xT_next[li + 1][:, :, b * P:(b + 1) * P],
                                in_=pt[:])
    nc.compile()
    return nc


def _prep_weights(W1, att1, W2, att2, W3, att3):
    """Host-side weight folding and layout prep (fp32 -> bf16)."""
    def fold_wj(W, att, H, C):
        return np.stack([att[h, C:] @ W[h * C:(h + 1) * C, :] for h in range(H)],
                        axis=1)  # [F_in, H]

    d = {}
    d["w1"] = np.ascontiguousarray(W1.T).astype(BF16)                # [128, 512]
    d["wj1"] = fold_wj(W1, att1, 4, 128).astype(BF16)                # [128, 4]
    d["w2"] = np.ascontiguousarray(W2.T).reshape(4, 128, 512).transpose(
        1, 0, 2).copy().astype(BF16)                                  # [128,4,512]
    d["wj2"] = fold_wj(W2, att2, 4, 128).reshape(4, 128, 4).transpose(
        1, 0, 2).copy().astype(BF16)                                  # [128,4,4]
    w3 = np.ascontiguousarray(W3.T).reshape(4, 128, 100)              # [4,128,100]
    wj3 = fold_wj(W3, att3, 1, 100).reshape(4, 128, 1)                # [4,128,1]
    d["w3"] = np.concatenate([w3, wj3], axis=2).transpose(
        1, 0, 2).copy().astype(BF16)                                  # [128,4,101]
    return d


_CACHE = {}


def _run(inputs, trace):
    from concourse.bass_utils import run_bass_kernel_spmd

    x = np.asarray(inputs["x"], np.float32)
    edge_index = np.asarray(inputs["edge_index"]).astype(np.int64)

    ep = _preprocess(edge_index)
    wd = _prep_weights(*[np.asarray(inputs[k], np.float32) for k in
                         ("W1", "att1", "W2", "att2", "W3", "att3")])

    ncore, nblk, npos = ep["ncore"], ep["nblk"], ep["npos"]
    lrow = nblk * P + npos
    x_slot = np.zeros((NCORES, NSHP, P), BF16)
    x_slot[ncore, lrow] = x.astype(BF16)

    key = ("prog", ep["T"], ep["NB"], ep["tb"].tobytes())
    if key not in _CACHE:
        _CACHE[key] = _build_program(ep)
    nc = _CACHE[key]

    in_maps = []
    for k in range(NCORES):
        m = dict(wd)
        m["xT"] = np.ascontiguousarray(x_slot[k].T)
        m["onehot"] = ep["onehot"][k]
        m["idxs"] = ep["idxs"][k]
        in_maps.append(m)

    res = run_bass_kernel_spmd(nc, in_maps, core_ids=list(range(NCORES)),
                               trace=trace)
    out_full = np.stack([res.results[k]["out"] for k in range(NCORES)])
    out = out_full[ncore, lrow].astype(np.float32)
    return out, res


def kernel(x, W1, att1, W2, att2, W3, att3, edge_index):
    out, _ = _run(dict(x=x, W1=W1, att1=att1, W2=W2, att2=att2, W3=W3,
                       att3=att3, edge_index=edge_index), trace=False)
    return out


def kernel_traced(inputs):
    return _run(inputs, trace=True)


# revision 11
# speedup vs baseline: 1.0612x; 1.0612x over previous
"""GAT (3-layer, no-LeakyReLU) on 8 Trainium2 NeuronCores — v3.

Math: softmax is separable (no LeakyReLU): with aj[n,h] = <h[n,h,:],
att[h,C:]> and u = exp(aj),
    out[d] = sum_{e: dst=d} u[src_e]*h[src_e] / sum_e u[src_e]
(the ai[dst] term cancels inside the per-destination softmax).

Per layer, per core (nodes sharded by destination):
  1. dense:  h = x @ W.T; u = exp(x @ wj); G row = [u*h | u] (640-wide)
  2. AllGather G -> replicated table in (pair-shared) HBM
  3. per dst-block of 128 nodes: dma_gather G[src] rows, one-hot matmul
     accumulates F = oh.T @ u*h and S1 = oh.T @ u
  4. out = F / S1; transpose to xT for the next layer's dense.

v3 over the v1 baseline:
  - Host LPT permutation balances in-edges per (core, dst-block) bin:
    tiles per layer drop 340 -> 320 and skew padding disappears.
  - L3 computes F and S1 in ONE 101-col matmul (row = [u*h | u]).
  - DVE ops fused: u*h and the output normalization use a single
    broadcast tensor_mul over [P,4,128] views; transposed blocks land in
    one [P,512] PSUM tile copied with one op.
  - Deeper gather pipeline (gt bufs 8).
"""

import numpy as np
import ml_dtypes

N = 20000
E = 320000
NCORES = 8
NSH = 2500            # real nodes per core
NSHP = 2560           # padded to 20 x 128
P = 128
RT = NSHP // P        # row tiles / dst blocks per core = 20
GB = 8                # gather batch: tiles per dma_gather (1024-desc scratch limit)

# layer configs: row = [u*h (H*C) | u (H)] padded to GW
L1 = dict(H=4, C=128, KB=1, GW=640, UO=512)
L2 = dict(H=4, C=128, KB=4, GW=640, UO=512)
L3 = dict(H=1, C=100, KB=4, GW=256, UO=100)

BF16 = ml_dtypes.bfloat16
F8 = ml_dtypes.float8_e4m3


def _balance_nodes(edge_index):
    """LPT: assign nodes to (core, block) bins balancing in-edge counts."""
    import heapq

    indeg = np.bincount(edge_index[1], minlength=N)
    order = np.argsort(-indeg, kind="stable")
    heap = [(0, 0, b) for b in range(NCORES * RT)]  # (edges, nnodes, bin)
    heapq.heapify(heap)
    node_bin = np.zeros(N, np.int64)
    node_pos = np.zeros(N, np.int64)
    for n in order:
        while True:
            e, c, b = heapq.heappop(heap)
            if c < P:
                break
        node_bin[n] = b
        node_pos[n] = c
        heapq.heappush(heap, (e + int(indeg[n]), c + 1, b))
    return node_bin // RT, node_bin % RT, node_pos


def _preprocess(edge_index):
    """Permute nodes; per-core gather indices + one-hot tiles grouped by
    dst block."""
    ncore, nblk, npos = _balance_nodes(edge_index)
    lrow = nblk * P + npos                       # local row within core

    src = edge_index[0].astype(np.int64)
    dst = edge_index[1].astype(np.int64)
    kd = ncore[dst]
    bd = nblk[dst]
    pd = npos[dst]
    rs = ncore[src] * NSHP + lrow[src]           # table row of source

    order = np.lexsort((rs, bd, kd))
    kd_s, bd_s, pd_s, rs_s = (a[order] for a in (kd, bd, pd, rs))

    cnt = np.zeros((NCORES, RT), np.int64)
    for k in range(NCORES):
        cnt[k] = np.bincount(bd_s[kd_s == k], minlength=RT)

    tb = np.ceil(cnt / P).astype(np.int64).max(axis=0)    # [RT]
    T = int(tb.sum())
    NB = (T + GB - 1) // GB

    block_of_tile = []
    first = []
    last = []
    grp_off = np.zeros(RT, np.int64)
    acc = 0
    for b in range(RT):
        grp_off[b] = acc
        for t in range(tb[b]):
            block_of_tile.append(b)
            first.append(t == 0)
            last.append(t == tb[b] - 1)
        acc += tb[b]
    block_of_tile = np.array(block_of_tile)
    first = np.array(first)
    last = np.array(last)

    idxs_all = np.zeros((NCORES, T * P), np.int64)
    onehot_all = np.zeros((NCORES, T, P, P), F8)
    for k in range(NCORES):
        m = kd_s == k
        bk, pk, rk = bd_s[m], pd_s[m], rs_s[m]
        off = np.concatenate([[0], np.cumsum(cnt[k])])
        for b in range(RT):
            e0, e1 = off[b], off[b + 1]
            n_e = e1 - e0
            if n_e == 0:
                continue
            slots = grp_off[b] * P + np.arange(n_e)
            idxs_all[k, slots] = rk[e0:e1]
            onehot_all[k, slots // P, slots % P, pk[e0:e1]] = 1.0

    gbc = GB * P // 16
    idx_wrapped = np.zeros((NCORES, 16, NB * gbc), np.int16)
    for g in range(NB):
        i0 = g * GB * P
        n_i = min(GB * P, T * P - i0)
        chunk = idxs_all[:, i0:i0 + n_i].astype(np.int16)
        idx_wrapped[:, :, g * gbc: g * gbc + n_i // 16] = (
            chunk.reshape(NCORES, n_i // 16, 16).transpose(0, 2, 1)
        )
    idx_rep = np.tile(idx_wrapped, (1, 8, 1))

    oh_b = np.zeros((NCORES, NB, P, GB * P), F8)
    for g in range(NB):
        nt = min(GB, T - g * GB)
        chunk = onehot_all[:, g * GB:g * GB + nt]
        oh_b[:, g, :, :nt * P] = chunk.transpose(0, 2, 1, 3).reshape(
            NCORES, P, nt * P)

    return dict(
        T=T, NB=NB, tb=tb,
        block_of_tile=block_of_tile, first=first, last=last,
        idxs=idx_rep, onehot=oh_b,
        ncore=ncore, nblk=nblk, npos=npos,
    )


def _build_program(ep):
    import concourse.bacc as bacc
    import concourse.mybir as mybir
    import concourse.tile as tile
    from concourse.masks import make_identity

    T, NB = ep["T"], ep["NB"]
    bot, first, last = ep["block_of_tile"], ep["first"], ep["last"]
    f32, bf16, i16 = mybir.dt.float32, mybir.dt.bfloat16, mybir.dt.int16
    f8 = mybir.dt.float8e4
    gbc = GB * P // 16

    nc = bacc.Bacc("TRN2", target_bir_lowering=False, debug=False,
                   num_devices=NCORES, num_swdge_queues=4)

    # ---- I/O ----
    xT_in = nc.dram_tensor("xT", [P, NSHP], bf16, kind="ExternalInput")
    w1_in = nc.dram_tensor("w1", [P, 512], bf16, kind="ExternalInput")
    wj1_in = nc.dram_tensor("wj1", [P, 4], bf16, kind="ExternalInput")
    w2_in = nc.dram_tensor("w2", [P, 4, 512], bf16, kind="ExternalInput")
    wj2_in = nc.dram_tensor("wj2", [P, 4, 4], bf16, kind="ExternalInput")
    w3_in = nc.dram_tensor("w3", [P, 4, 101], bf16, kind="ExternalInput")
    oh_in = nc.dram_tensor("onehot", [NB, P, GB * P], f8,
                           kind="ExternalInput")
    idx_in = nc.dram_tensor("idxs", [P, NB * gbc], i16, kind="ExternalInput")
    out_d = nc.dram_tensor("out", [NSHP, 100], f32, kind="ExternalOutput")

    # ---- internal DRAM ----
    ag = [nc.dram_tensor(f"ag{i}", [NSHP, L["GW"]], bf16)
          for i, L in enumerate((L1, L2, L3))]
    table = [nc.dram_tensor(f"table{i}", [NCORES * NSHP, L["GW"]], bf16,
                            addr_space="Shared")
             for i, L in enumerate((L1, L2, L3))]

    with tile.TileContext(nc, num_cores=NCORES) as tc:
        with (
            tc.tile_pool(name="const", bufs=1) as cp,
            tc.tile_pool(name="sb", bufs=2) as sb,
            tc.tile_pool(name="gat", bufs=8) as gp,
            tc.tile_pool(name="small", bufs=4) as sp,
            tc.tile_pool(name="psum", bufs=2, space="PSUM") as psA,
        ):
            ident = cp.tile([P, P], bf16, tag="ident")
            make_identity(nc, ident[:])
            idx_sb = cp.tile([P, NB * gbc], i16, tag="idx")
            nc.sync.dma_start(out=idx_sb[:], in_=idx_in[:, :])
            w_sb = []
            wj_sb = []
            for i, (L, wt, wj) in enumerate(
                ((L1, w1_in, wj1_in), (L2, w2_in, wj2_in), (L3, w3_in, None))
            ):
                nout = 512 if i < 2 else 101
                w = cp.tile([P, L["KB"], nout], bf16, tag=f"w{i}", name=f"w{i}")
                nc.sync.dma_start(out=w[:, 0, :] if i == 0 else w[:],
                                  in_=wt.ap())
                w_sb.append(w)
                if wj is not None:
                    wjt = cp.tile([P, L["KB"], L["H"]], bf16, tag=f"wj{i}",
                                  name=f"wj{i}")
                    nc.sync.dma_start(out=wjt[:, 0, :] if i == 0 else wjt[:],
                                      in_=wj.ap())
                    wj_sb.append(wjt)
                else:
                    wj_sb.append(None)

            xT1 = cp.tile([P, 1, NSHP], bf16, tag="xT1")
            nc.sync.dma_start(out=xT1[:, 0, :], in_=xT_in.ap())
            xT2 = cp.tile([P, 4, NSHP], bf16, tag="xT2", name="xT2")
            xT3 = cp.tile([P, 4, NSHP], bf16, tag="xT3", name="xT3")
            xT_next = [None, xT2, xT3]

            for li, L in enumerate((L1, L2, L3)):
                H, C, KB, GW, UO = L["H"], L["C"], L["KB"], L["GW"], L["UO"]
                is_l3 = li == 2
                G_sb = sb.tile([P, RT, GW], bf16, tag="G", bufs=1)
                xT = xT1 if li == 0 else xT_next[li]

                # ---------- dense + u ----------
                for r in range(RT):
                    if is_l3:
                        # single matmul: w3 holds [W | wj] = 101 cols
                        ph = psA.tile([P, 101], f32, tag="ph")
                        for kb in range(KB):
                            nc.tensor.matmul(ph[:], xT[:, kb, r * P:(r + 1) * P],
                                             w_sb[li][:, kb, :],
                                             start=(kb == 0),
                                             stop=(kb == KB - 1))
                        u = sp.tile([P, 1], f32, tag="u")
                        nc.scalar.activation(u[:], ph[:, 100:101],
                                             mybir.ActivationFunctionType.Exp)
                        nc.vector.tensor_scalar_mul(
                            G_sb[:, r, 0:100], ph[:, 0:100], u[:, 0:1])
                        nc.vector.tensor_copy(out=G_sb[:, r, 100:101],
                                              in_=u[:])
                    else:
                        ph = psA.tile([P, 512], f32, tag="ph")
                        pa = psA.tile([P, H], f32, tag="pa", bufs=1)
                        for kb in range(KB):
                            lhsT = xT[:, kb, r * P:(r + 1) * P]
                            nc.tensor.matmul(ph[:], lhsT, w_sb[li][:, kb, :],
                                             start=(kb == 0),
                                             stop=(kb == KB - 1))
                            nc.tensor.matmul(pa[:], lhsT, wj_sb[li][:, kb, :],
                                             start=(kb == 0),
                                             stop=(kb == KB - 1))
                        u = sp.tile([P, H], f32, tag="u")
                        nc.scalar.activation(u[:], pa[:],
                                             mybir.ActivationFunctionType.Exp)
                        # u*h in ONE broadcast multiply over [P,H,C]
                        nc.vector.tensor_mul(
                            out=G_sb[:, r, 0:512].rearrange(
                                "p (h c) -> p h c", c=C),
                            in0=ph[:].rearrange("p (h c) -> p h c", c=C),
                            in1=u[:].unsqueeze(2).to_broadcast([P, H, C]))
                        nc.vector.tensor_copy(out=G_sb[:, r, UO:UO + H],
                                              in_=u[:])
                    nc.sync.dma_start(
                        out=ag[li][r * P:(r + 1) * P, :],
                        in_=G_sb[:, r, :])

                # ---------- exchange ----------
                nc.gpsimd.collective_compute(
                    "AllGather", mybir.AluOpType.bypass,
                    replica_groups=[list(range(NCORES))],
                    ins=[ag[li].ap().opt()],
                    outs=[table[li].ap().opt()])

                # ---------- edge aggregation ----------
                pF = pS = None
                for g in range(NB):
                    s0 = g * GB
                    nt = min(GB, T - s0)
                    gt = gp.tile([P, GB, GW], bf16, tag="gt")
                    nc.gpsimd.dma_gather(
                        gt[:, :nt, :], table[li].ap(),
                        idx_sb[:, g * gbc: g * gbc + nt * P // 16],
                        nt * P, nt * P, GW, queue_num=g % 4)
                    oh = gp.tile([P, GB, P], f8, tag="oh", bufs=10)
                    nc.sync.dma_start(
                        out=oh[:, :nt, :],
                        in_=oh_in[g, :, :nt * P].rearrange(
                            "p (t d) -> p t d", d=P))
                    for tl in range(nt):
                        ti = s0 + tl
                        b = bot[ti]
                        if first[ti]:
                            pF = psA.tile([P, 101 if is_l3 else 512], f32,
                                          tag="pF")
                            if not is_l3:
                                pS = psA.tile([P, H], f32, tag="pS")
                        if is_l3:
                            nc.tensor.matmul(pF[:], oh[:, tl, :],
                                             gt[:, tl, 0:101],
                                             start=bool(first[ti]),
                                             stop=bool(last[ti]))
                        else:
                            nc.tensor.matmul(pF[:], oh[:, tl, :],
                                             gt[:, tl, 0:512],
                                             start=bool(first[ti]),
                                             stop=bool(last[ti]))
                            nc.tensor.matmul(pS[:], oh[:, tl, :],
                                             gt[:, tl, UO:UO + H],
                                             start=bool(first[ti]),
                                             stop=bool(last[ti]))
                        if not last[ti]:
                            continue
                        # ---------- block evacuation ----------
                        if is_l3:
                            nc.vector.tensor_add(
                                out=pF[:], in0=pF[:],
                                in1=G_sb[:, b, 0:101])          # self loop
                            s1c = sp.tile([P, 1], f32, tag="s1c")
                            nc.vector.tensor_scalar_max(
                                s1c[:], pF[:, 100:101], 1e-30)
                            rec = sp.tile([P, 1], f32, tag="rec")
                            nc.vector.reciprocal(rec[:], s1c[:])
                            o3 = sb.tile([P, 100], f32, tag="o3")
                            nc.vector.tensor_scalar_mul(
                                o3[:], pF[:, 0:100], rec[:, 0:1])
                            nc.sync.dma_start(
                                out=out_d[b * P:(b + 1) * P, :], in_=o3[:])
                        else:
                            nc.vector.tensor_add(
                                out=pF[:], in0=pF[:],
                                in1=G_sb[:, b, 0:512])          # self loop
                            nc.vector.tensor_add(
                                out=pS[:], in0=pS[:],
                                in1=G_sb[:, b, UO:UO + H])
                            s1c = sp.tile([P, H], f32, tag="s1c")
                            nc.vector.tensor_scalar_max(s1c[:], pS[:], 1e-30)
                            rec = sp.tile([P, H], f32, tag="rec")
                            nc.vector.reciprocal(rec[:], s1c[:])
                            ob = sb.tile([P, 512], bf16, tag="ob")
                            nc.vector.tensor_mul(
                                out=ob[:].rearrange("p (h c) -> p h c", c=C),
                                in0=pF[:].rearrange("p (h c) -> p h c", c=C),
                                in1=rec[:].unsqueeze(2).to_broadcast(
                                    [P, H, C]))
                            pt = psA.tile([P, 4, P], bf16, tag="pt", bufs=1)
                            for fb in range(4):
                                nc.tensor.transpose(
                                    pt[:, fb, :], ob[:, fb * P:(fb + 1) * P],
                                    ident[:])
                            nc.vector.tensor_copy(
                                out=xT_next[li + 1][:, :, b * P:(b + 1) * P],
                                in_=pt[:])
    nc.compile()
    return nc


def _prep_weights(W1, att1, W2, att2, W3, att3):
    """Host-side weight folding and layout prep (fp32 -> bf16)."""
    def fold_wj(W, att, H, C):
        return np.stack([att[h, C:] @ W[h * C:(h + 1) * C, :] for h in range(H)],
                        axis=1)  # [F_in, H]

    d = {}
    d["w1"] = np.ascontiguousarray(W1.T).astype(BF16)                # [128, 512]
    d["wj1"] = fold_wj(W1, att1, 4, 128).astype(BF16)                # [128, 4]
    d["w2"] = np.ascontiguousarray(W2.T).reshape(4, 128, 512).transpose(
        1, 0, 2).copy().astype(BF16)                                  # [128,4,512]
    d["wj2"] = fold_wj(W2, att2, 4, 128).reshape(4, 128, 4).transpose(
        1, 0, 2).copy().astype(BF16)                                  # [128,4,4]
    w3 = np.ascontiguousarray(W3.T).reshape(4, 128, 100)              # [4,128,100]
    wj3 = fold_wj(W3, att3, 1, 100).reshape(4, 128, 1)                # [4,128,1]
    d["w3"] = np.concatenate([w3, wj3], axis=2).transpose(
        1, 0, 2).copy().astype(BF16)                                  # [128,4,101]
    return d


_CACHE = {}


def _run(inputs, trace):
    from concourse.bass_utils import run_bass_kernel_spmd

    x = np.asarray(inputs["x"], np.float32)
    edge_index = np.asarray(inputs["edge_index"]).astype(np.int64)

    ep = _preprocess(edge_index)
    wd = _prep_weights(*[np.asarray(inputs[k], np.float32) for k in
                         ("W1", "att1", "W2", "att2", "W3", "att3")])

    ncore, nblk, npos = ep["ncore"], ep["nblk"], ep["npos"]
    lrow = nblk * P + npos
    x_slot = np.zeros((NCORES, NSHP, P), BF16)
    x_slot[ncore, lrow] = x.astype(BF16)

    key = ("prog", ep["T"], ep["NB"], ep["tb"].tobytes())
    if key not in _CACHE:
        _CACHE[key] = _build_program(ep)
    nc = _CACHE[key]

    in_maps = []
    for k in range(NCORES):
        m = dict(wd)
        m["xT"] = np.ascontiguousarray(x_slot[k].T)
        m["onehot"] = ep["onehot"][k]
        m["idxs"] = ep["idxs"][k]
        in_maps.append(m)

    res = run_bass_kernel_spmd(nc, in_maps, core_ids=list(range(NCORES)),
                               trace=trace)
    out_full = np.stack([res.results[k]["out"] for k in range(NCORES)])
    out = out_full[ncore, lrow].astype(np.float32)
    return out, res


def kernel(x, W1, att1, W2, att2, W3, att3, edge_index):
    out, _ = _run(dict(x=x, W1=W1, att1=att1, W2=W2, att2=att2, W3=W3,
                       att3=att3, edge_index=edge_index), trace=False)
    return out


def kernel_traced(inputs):
    return _run(inputs, trace=True)


# revision 13
# speedup vs baseline: 1.0636x; 1.0023x over previous
"""GAT (3-layer, no-LeakyReLU) on 8 Trainium2 NeuronCores — v3.

Math: softmax is separable (no LeakyReLU): with aj[n,h] = <h[n,h,:],
att[h,C:]> and u = exp(aj),
    out[d] = sum_{e: dst=d} u[src_e]*h[src_e] / sum_e u[src_e]
(the ai[dst] term cancels inside the per-destination softmax).

Per layer, per core (nodes sharded by destination):
  1. dense:  h = x @ W.T; u = exp(x @ wj); G row = [u*h | u] (640-wide)
  2. AllGather G -> replicated table in (pair-shared) HBM
  3. per dst-block of 128 nodes: dma_gather G[src] rows, one-hot matmul
     accumulates F = oh.T @ u*h and S1 = oh.T @ u
  4. out = F / S1; transpose to xT for the next layer's dense.

v3 over the v1 baseline:
  - Host LPT permutation balances in-edges per (core, dst-block) bin:
    tiles per layer drop 340 -> 320 and skew padding disappears.
  - L3 computes F and S1 in ONE 101-col matmul (row = [u*h | u]).
  - DVE ops fused: u*h and the output normalization use a single
    broadcast tensor_mul over [P,4,128] views; transposed blocks land in
    one [P,512] PSUM tile copied with one op.
  - Deeper gather pipeline (gt bufs 8).
"""

import numpy as np
import ml_dtypes

N = 20000
E = 320000
NCORES = 8
NSH = 2500            # real nodes per core
NSHP = 2560           # padded to 20 x 128
P = 128
RT = NSHP // P        # row tiles / dst blocks per core = 20
GB = 8                # gather batch: tiles per dma_gather (1024-desc scratch limit)

# layer configs: row = [u*h (H*C) | u (H)] padded to GW
L1 = dict(H=4, C=128, KB=1, GW=640, UO=512)
L2 = dict(H=4, C=128, KB=4, GW=640, UO=512)
L3 = dict(H=1, C=100, KB=4, GW=256, UO=100)

BF16 = ml_dtypes.bfloat16
F8 = ml_dtypes.float8_e4m3


def _balance_nodes(edge_index):
    """LPT: assign nodes to (core, block) bins balancing in-edge counts."""
    import heapq

    indeg = np.bincount(edge_index[1], minlength=N)
    order = np.argsort(-indeg, kind="stable")
    heap = [(0, 0, b) for b in range(NCORES * RT)]  # (edges, nnodes, bin)
    heapq.heapify(heap)
    node_bin = np.zeros(N, np.int64)
    node_pos = np.zeros(N, np.int64)
    for n in order:
        while True:
            e, c, b = heapq.heappop(heap)
            if c < P:
                break
        node_bin[n] = b
        node_pos[n] = c
        heapq.heappush(heap, (e + int(indeg[n]), c + 1, b))
    return node_bin // RT, node_bin % RT, node_pos


def _preprocess(edge_index):
    """Permute nodes; per-core gather indices + one-hot tiles grouped by
    dst block."""
    ncore, nblk, npos = _balance_nodes(edge_index)
    lrow = nblk * P + npos                       # local row within core

    src = edge_index[0].astype(np.int64)
    dst = edge_index[1].astype(np.int64)
    kd = ncore[dst]
    bd = nblk[dst]
    pd = npos[dst]
    rs = ncore[src] * NSHP + lrow[src]           # table row of source

    order = np.lexsort((rs, bd, kd))
    kd_s, bd_s, pd_s, rs_s = (a[order] for a in (kd, bd, pd, rs))

    cnt = np.zeros((NCORES, RT), np.int64)
    for k in range(NCORES):
        cnt[k] = np.bincount(bd_s[kd_s == k], minlength=RT)

    tb = np.ceil(cnt / P).astype(np.int64).max(axis=0)    # [RT]
    T = int(tb.sum())
    NB = (T + GB - 1) // GB

    block_of_tile = []
    first = []
    last = []
    grp_off = np.zeros(RT, np.int64)
    acc = 0
    for b in range(RT):
        grp_off[b] = acc
        for t in range(tb[b]):
            block_of_tile.append(b)
            first.append(t == 0)
            last.append(t == tb[b] - 1)
        acc += tb[b]
    block_of_tile = np.array(block_of_tile)
    first = np.array(first)
    last = np.array(last)

    idxs_all = np.zeros((NCORES, T * P), np.int64)
    onehot_all = np.zeros((NCORES, T, P, P), F8)
    for k in range(NCORES):
        m = kd_s == k
        bk, pk, rk = bd_s[m], pd_s[m], rs_s[m]
        off = np.concatenate([[0], np.cumsum(cnt[k])])
        for b in range(RT):
            e0, e1 = off[b], off[b + 1]
            n_e = e1 - e0
            if n_e == 0:
                continue
            slots = grp_off[b] * P + np.arange(n_e)
            idxs_all[k, slots] = rk[e0:e1]
            onehot_all[k, slots // P, slots % P, pk[e0:e1]] = 1.0

    gbc = GB * P // 16
    idx_wrapped = np.zeros((NCORES, 16, NB * gbc), np.int16)
    for g in range(NB):
        i0 = g * GB * P
        n_i = min(GB * P, T * P - i0)
        chunk = idxs_all[:, i0:i0 + n_i].astype(np.int16)
        idx_wrapped[:, :, g * gbc: g * gbc + n_i // 16] = (
            chunk.reshape(NCORES, n_i // 16, 16).transpose(0, 2, 1)
        )
    idx_rep = np.tile(idx_wrapped, (1, 8, 1))

    oh_b = np.zeros((NCORES, NB, P, GB * P), F8)
    for g in range(NB):
        nt = min(GB, T - g * GB)
        chunk = onehot_all[:, g * GB:g * GB + nt]
        oh_b[:, g, :, :nt * P] = chunk.transpose(0, 2, 1, 3).reshape(
            NCORES, P, nt * P)

    return dict(
        T=T, NB=NB, tb=tb,
        block_of_tile=block_of_tile, first=first, last=last,
        idxs=idx_rep, onehot=oh_b,
        ncore=ncore, nblk=nblk, npos=npos,
    )


def _build_program(ep):
    import concourse.bacc as bacc
    import concourse.mybir as mybir
    import concourse.tile as tile
    from concourse.masks import make_identity

    T, NB = ep["T"], ep["NB"]
    bot, first, last = ep["block_of_tile"], ep["first"], ep["last"]
    f32, bf16, i16 = mybir.dt.float32, mybir.dt.bfloat16, mybir.dt.int16
    f8 = mybir.dt.float8e4
    gbc = GB * P // 16

    nc = bacc.Bacc("TRN2", target_bir_lowering=False, debug=False,
                   num_devices=NCORES, num_swdge_queues=4)

    # ---- I/O ----
    xT_in = nc.dram_tensor("xT", [P, NSHP], bf16, kind="ExternalInput")
    w1_in = nc.dram_tensor("w1", [P, 512], bf16, kind="ExternalInput")
    wj1_in = nc.dram_tensor("wj1", [P, 4], bf16, kind="ExternalInput")
    w2_in = nc.dram_tensor("w2", [P, 4, 512], bf16, kind="ExternalInput")
    wj2_in = nc.dram_tensor("wj2", [P, 4, 4], bf16, kind="ExternalInput")
    w3_in = nc.dram_tensor("w3", [P, 4, 101], bf16, kind="ExternalInput")
    oh_in = nc.dram_tensor("onehot", [NB, P, GB * P], f8,
                           kind="ExternalInput")
    idx_in = nc.dram_tensor("idxs", [P, NB * gbc], i16, kind="ExternalInput")
    out_d = nc.dram_tensor("out", [NSHP, 100], f32, kind="ExternalOutput")

    # ---- internal DRAM ----
    ag = [nc.dram_tensor(f"ag{i}", [NSHP, L["GW"]], bf16)
          for i, L in enumerate((L1, L2, L3))]
    table = [nc.dram_tensor(f"table{i}", [NCORES * NSHP, L["GW"]], bf16,
                            addr_space="Shared")
             for i, L in enumerate((L1, L2, L3))]

    with tile.TileContext(nc, num_cores=NCORES) as tc:
        with (
            tc.tile_pool(name="const", bufs=1) as cp,
            tc.tile_pool(name="sb", bufs=2) as sb,
            tc.tile_pool(name="gat", bufs=8) as gp,
            tc.tile_pool(name="small", bufs=4) as sp,
            tc.tile_pool(name="psum", bufs=2, space="PSUM") as psA,
        ):
            ident = cp.tile([P, P], bf16, tag="ident")
            make_identity(nc, ident[:])
            idx_sb = cp.tile([P, NB * gbc], i16, tag="idx")
            nc.sync.dma_start(out=idx_sb[:], in_=idx_in[:, :])
            w_sb = []
            wj_sb = []
            for i, (L, wt, wj) in enumerate(
                ((L1, w1_in, wj1_in), (L2, w2_in, wj2_in), (L3, w3_in, None))
            ):
                nout = 512 if i < 2 else 101
                w = cp.tile([P, L["KB"], nout], bf16, tag=f"w{i}", name=f"w{i}")
                nc.sync.dma_start(out=w[:, 0, :] if i == 0 else w[:],
                                  in_=wt.ap())
                w_sb.append(w)
                if wj is not None:
                    wjt = cp.tile([P, L["KB"], L["H"]], bf16, tag=f"wj{i}",
                                  name=f"wj{i}")
                    nc.sync.dma_start(out=wjt[:, 0, :] if i == 0 else wjt[:],
                                      in_=wj.ap())
                    wj_sb.append(wjt)
                else:
                    wj_sb.append(None)

            xT1 = cp.tile([P, 1, NSHP], bf16, tag="xT1")
            nc.sync.dma_start(out=xT1[:, 0, :], in_=xT_in.ap())
            xT2 = cp.tile([P, 4, NSHP], bf16, tag="xT2", name="xT2")
            xT3 = cp.tile([P, 4, NSHP], bf16, tag="xT3", name="xT3")
            xT_next = [None, xT2, xT3]

            for li, L in enumerate((L1, L2, L3)):
                H, C, KB, GW, UO = L["H"], L["C"], L["KB"], L["GW"], L["UO"]
                is_l3 = li == 2
                G_sb = sb.tile([P, RT, GW], bf16, tag="G", bufs=1)
                xT = xT1 if li == 0 else xT_next[li]

                # ---------- dense + u ----------
                for r in range(RT):
                    if is_l3:
                        # single matmul: w3 holds [W | wj] = 101 cols
                        ph = psA.tile([P, 101], f32, tag="ph")
                        for kb in range(KB):
                            nc.tensor.matmul(ph[:], xT[:, kb, r * P:(r + 1) * P],
                                             w_sb[li][:, kb, :],
                                             start=(kb == 0),
                                             stop=(kb == KB - 1))
                        u = sp.tile([P, 1], f32, tag="u")
                        nc.scalar.activation(u[:], ph[:, 100:101],
                                             mybir.ActivationFunctionType.Exp)
                        nc.vector.tensor_scalar_mul(
                            G_sb[:, r, 0:100], ph[:, 0:100], u[:, 0:1])
                        nc.vector.tensor_copy(out=G_sb[:, r, 100:101],
                                              in_=u[:])
                    else:
                        ph = psA.tile([P, 512], f32, tag="ph")
                        pa = psA.tile([P, H], f32, tag="pa", bufs=1)
                        for kb in range(KB):
                            lhsT = xT[:, kb, r * P:(r + 1) * P]
                            nc.tensor.matmul(ph[:], lhsT, w_sb[li][:, kb, :],
                                             start=(kb == 0),
                                             stop=(kb == KB - 1))
                            nc.tensor.matmul(pa[:], lhsT, wj_sb[li][:, kb, :],
                                             start=(kb == 0),
                                             stop=(kb == KB - 1))
                        u = sp.tile([P, H], f32, tag="u")
                        nc.scalar.activation(u[:], pa[:],
                                             mybir.ActivationFunctionType.Exp)
                        # u*h in ONE broadcast multiply over [P,H,C]
                        nc.vector.tensor_mul(
                            out=G_sb[:, r, 0:512].rearrange(
                                "p (h c) -> p h c", c=C),
                            in0=ph[:].rearrange("p (h c) -> p h c", c=C),
                            in1=u[:].unsqueeze(2).to_broadcast([P, H, C]))
                        nc.vector.tensor_copy(out=G_sb[:, r, UO:UO + H],
                                              in_=u[:])
                    nc.sync.dma_start(
                        out=ag[li][r * P:(r + 1) * P, :],
                        in_=G_sb[:, r, :])

                # ---------- exchange ----------
                nc.gpsimd.collective_compute(
                    "AllGather", mybir.AluOpType.bypass,
                    replica_groups=[list(range(NCORES))],
                    ins=[ag[li].ap().opt()],
                    outs=[table[li].ap().opt()])

                # ---------- edge aggregation ----------
                pF = pS = None
                for g in range(NB):
                    s0 = g * GB
                    nt = min(GB, T - s0)
                    gt = gp.tile([P, GB, GW], bf16, tag="gt", bufs=10)
                    nc.gpsimd.dma_gather(
                        gt[:, :nt, :], table[li].ap(),
                        idx_sb[:, g * gbc: g * gbc + nt * P // 16],
                        nt * P, nt * P, GW, queue_num=g % 4)
                    oh = gp.tile([P, GB, P], f8, tag="oh", bufs=10)
                    nc.sync.dma_start(
                        out=oh[:, :nt, :],
                        in_=oh_in[g, :, :nt * P].rearrange(
                            "p (t d) -> p t d", d=P))
                    for tl in range(nt):
                        ti = s0 + tl
                        b = bot[ti]
                        if first[ti]:
                            pF = psA.tile([P, 101 if is_l3 else 512], f32,
                                          tag="pF")
                            if not is_l3:
                                pS = psA.tile([P, H], f32, tag="pS")
                        if is_l3:
                            nc.tensor.matmul(pF[:], oh[:, tl, :],
                                             gt[:, tl, 0:101],
                                             start=bool(first[ti]),
                                             stop=bool(last[ti]))
                        else:
                            nc.tensor.matmul(pF[:], oh[:, tl, :],
                                             gt[:, tl, 0:512],
                                             start=bool(first[ti]),
                                             stop=bool(last[ti]))
                            nc.tensor.matmul(pS[:], oh[:, tl, :],
                                             gt[:, tl, UO:UO + H],
                                             start=bool(first[ti]),
                                             stop=bool(last[ti]))
                        if not last[ti]:
                            continue
                        # ---------- block evacuation ----------
                        if is_l3:
                            nc.vector.tensor_add(
                                out=pF[:], in0=pF[:],
                                in1=G_sb[:, b, 0:101])          # self loop
                            s1c = sp.tile([P, 1], f32, tag="s1c")
                            nc.vector.tensor_scalar_max(
                                s1c[:], pF[:, 100:101], 1e-30)
                            rec = sp.tile([P, 1], f32, tag="rec")
                            nc.vector.reciprocal(rec[:], s1c[:])
                            o3 = sb.tile([P, 100], f32, tag="o3")
                            nc.vector.tensor_scalar_mul(
                                o3[:], pF[:, 0:100], rec[:, 0:1])
                            nc.sync.dma_start(
                                out=out_d[b * P:(b + 1) * P, :], in_=o3[:])
                        else:
                            nc.vector.tensor_add(
                                out=pF[:], in0=pF[:],
                                in1=G_sb[:, b, 0:512])          # self loop
                            nc.vector.tensor_add(
                                out=pS[:], in0=pS[:],
                                in1=G_sb[:, b, UO:UO + H])
                            s1c = sp.tile([P, H], f32, tag="s1c")
                            nc.vector.tensor_scalar_max(s1c[:], pS[:], 1e-30)
                            rec = sp.tile([P, H], f32, tag="rec")
                            nc.vector.reciprocal(rec[:], s1c[:])
                            ob = sb.tile([P, 512], bf16, tag="ob")
                            nc.vector.tensor_mul(
                                out=ob[:].rearrange("p (h c) -> p h c", c=C),
                                in0=pF[:].rearrange("p (h c) -> p h c", c=C),
                                in1=rec[:].unsqueeze(2).to_broadcast(
                                    [P, H, C]))
                            pt = psA.tile([P, 4, P], bf16, tag="pt", bufs=1)
                            for fb in range(4):
                                nc.tensor.transpose(
                                    pt[:, fb, :], ob[:, fb * P:(fb + 1) * P],
                                    ident[:])
                            nc.vector.tensor_copy(
                                out=xT_next[li + 1][:, :, b * P:(b + 1) * P],
                                in_=pt[:])
    nc.compile()
    return nc


def _prep_weights(W1, att1, W2, att2, W3, att3):
    """Host-side weight folding and layout prep (fp32 -> bf16)."""
    def fold_wj(W, att, H, C):
        return np.stack([att[h, C:] @ W[h * C:(h + 1) * C, :] for h in range(H)],
                        axis=1)  # [F_in, H]

    d = {}
    d["w1"] = np.ascontiguousarray(W1.T).astype(BF16)                # [128, 512]
    d["wj1"] = fold_wj(W1, att1, 4, 128).astype(BF16)                # [128, 4]
    d["w2"] = np.ascontiguousarray(W2.T).reshape(4, 128, 512).transpose(
        1, 0, 2).copy().astype(BF16)                                  # [128,4,512]
    d["wj2"] = fold_wj(W2, att2, 4, 128).reshape(4, 128, 4).transpose(
        1, 0, 2).copy().astype(BF16)                                  # [128,4,4]
    w3 = np.ascontiguousarray(W3.T).reshape(4, 128, 100)              # [4,128,100]
    wj3 = fold_wj(W3, att3, 1, 100).reshape(4, 128, 1)                # [4,128,1]
    d["w3"] = np.concatenate([w3, wj3], axis=2).transpose(
        1, 0, 2).copy().astype(BF16)                                  # [128,4,101]
    return d


_CACHE = {}


def _run(inputs, trace):
    from concourse.bass_utils import run_bass_kernel_spmd

    x = np.asarray(inputs["x"], np.float32)
    edge_index = np.asarray(inputs["edge_index"]).astype(np.int64)

    ep = _preprocess(edge_index)
    wd = _prep_weights(*[np.asarray(inputs[k], np.float32) for k in
                         ("W1", "att1", "W2", "att2", "W3", "att3")])

    ncore, nblk, npos = ep["ncore"], ep["nblk"], ep["npos"]
    lrow = nblk * P + npos
    x_slot = np.zeros((NCORES, NSHP, P), BF16)
    x_slot[ncore, lrow] = x.astype(BF16)

    key = ("prog", ep["T"], ep["NB"], ep["tb"].tobytes())
    if key not in _CACHE:
        _CACHE[key] = _build_program(ep)
    nc = _CACHE[key]

    in_maps = []
    for k in range(NCORES):
        m = dict(wd)
        m["xT"] = np.ascontiguousarray(x_slot[k].T)
        m["onehot"] = ep["onehot"][k]
        m["idxs"] = ep["idxs"][k]
        in_maps.append(m)

    res = run_bass_kernel_spmd(nc, in_maps, core_ids=list(range(NCORES)),
                               trace=trace)
    out_full = np.stack([res.results[k]["out"] for k in range(NCORES)])
    out = out_full[ncore, lrow].astype(np.float32)
    return out, res


def kernel(x, W1, att1, W2, att2, W3, att3, edge_index):
    out, _ = _run(dict(x=x, W1=W1, att1=att1, W2=W2, att2=att2, W3=W3,
                       att3=att3, edge_index=edge_index), trace=False)
    return out


def kernel_traced(inputs):
    return _run(inputs, trace=True)
